# revision 19
# baseline (speedup 1.0000x reference)
"""BertAttention Trainium2 kernel — 8-core SPMD.

Sharding: each core owns 2 heads (128 of the 1024 feature dims).
  - QKV projections for its heads over all tokens (batch-major token order)
  - attention for its 4 (batch, head) pairs
  - AllToAll exchanges ctx^T slices -> each core holds all 1024 ctx dims
    for its 512-token slice
  - output projection (full Wo) + residual + LayerNorm on its token slice
  - host concatenates per-core (512, 1024) outputs.

Host passes activations/weights pre-transposed and pre-cast to bf16
(feature-major), so the device spends no time on casts/transposes.
"""

import os
import sys

for _p in ("/opt/trn_rl_repo", "/root/.axon_site/_ro/trn_rl_repo"):
    if os.path.isdir(_p) and _p not in sys.path:
        sys.path.append(_p)

import ml_dtypes
import numpy as np

import concourse.bass as bass
import concourse.tile as tile
from concourse import bacc, mybir
from concourse.bass_utils import run_bass_kernel_spmd

F32 = mybir.dt.float32
BF16 = mybir.dt.bfloat16
BF16_NP = ml_dtypes.bfloat16

NCORES = 8
H = 16  # heads total
DH = 64  # head dim
LN_EPS = 1e-12


def build_bert_kernel(S=2048, B=2, D=1024, debug_dumps=False):
    P = 128
    NTOK = S * B              # batch-major tokens
    TPC = NTOK // NCORES      # tokens per core (output slice)
    CCH = D // P              # contraction chunks (8)
    HPC = H // NCORES         # heads per core (2)
    DL = HPC * DH             # local feature dims (128)
    NI = S // 512             # i-chunks per batch (512 queries each)
    NJ = S // P               # j-chunks per batch (128 keys each)
    JG = 2                    # j-chunks per exp group
    NVT = NTOK // P           # v token tiles
    NI2 = TPC // P            # out-proj token tiles per core

    nc = bacc.Bacc("TRN2", target_bir_lowering=False, debug=False,
                   num_devices=NCORES)

    def din(name, shape, dt=F32):
        return nc.dram_tensor(name, list(shape), dt, kind="ExternalInput").ap()

    xqT = din("xqT", (D, NTOK), BF16)
    xkT = din("xkT", (D, NTOK), BF16)
    xvT = din("xvT", (D, NTOK), BF16)
    wqT = din("wqT", (D, DL), BF16)
    wkT = din("wkT", (D, DL), BF16)
    wvT = din("wvT", (D, DL), BF16)
    woT = din("woT", (D, D), BF16)
    bq = din("bq", (DL, 1))
    bk = din("bk", (DL, 1))
    bv = din("bv", (1, DL))
    bo = din("bo", (1, D))
    lnw = din("lnw", (1, D))
    lnb = din("lnb", (1, D))
    resid = din("resid", (TPC, D))
    out = nc.dram_tensor("out", [TPC, D], F32, kind="ExternalOutput").ap()

    a2a_in = nc.dram_tensor("a2a_in", [NCORES, P, TPC], BF16).ap()
    ag_outs = nc.dram_tensor("ag_outs", [NCORES, D, TPC], BF16).ap()

    with tile.TileContext(nc) as tc:
        with (
            tc.tile_pool(name="persist", bufs=1) as persist,
            tc.tile_pool(name="small", bufs=1) as small,
        ):
            # ---- weights into SBUF (plain DMA, already transposed) ----
            wqT_sb = persist.tile([P, CCH, DL], BF16)
            wkT_sb = persist.tile([P, CCH, DL], BF16)
            wvT_sb = persist.tile([P, CCH, DL], BF16)
            w_engs = (nc.sync, nc.scalar, nc.gpsimd)
            for wi, (w_d, w_sb) in enumerate(
                    ((wqT, wqT_sb), (wkT, wkT_sb), (wvT, wvT_sb))):
                for c in range(CCH):
                    w_engs[wi].dma_start(out=w_sb[:, c, :],
                                         in_=w_d[c * P:(c + 1) * P, :])
            woT_sb = persist.tile([P, CCH, D], BF16)

            # ---- constant / bias tiles ----
            bq_sb = small.tile([P, 1], F32)
            bk_sb = small.tile([P, 1], F32)
            nc.sync.dma_start(out=bq_sb, in_=bq)
            nc.sync.dma_start(out=bk_sb, in_=bk)
            bv_bc = small.tile([P, DL], F32)
            nc.gpsimd.dma_start(out=bv_bc, in_=bv.to_broadcast((P, DL)))
            bo_bc = small.tile([P, D], F32)
            nc.gpsimd.dma_start(out=bo_bc, in_=bo.to_broadcast((P, D)))
            lnw_bc = small.tile([P, D], F32)
            nc.gpsimd.dma_start(out=lnw_bc, in_=lnw.to_broadcast((P, D)))
            lnb_bc = small.tile([P, D], F32)
            nc.gpsimd.dma_start(out=lnb_bc, in_=lnb.to_broadcast((P, D)))
            eps_sb = small.tile([P, 1], F32)
            nc.vector.memset(eps_sb, LN_EPS)

            # ---- persistent activation buffers ----
            qT_sb = persist.tile([P, NTOK], BF16)   # [dloc, tok]
            kT_sb = persist.tile([P, NTOK], BF16)
            v_sb = persist.tile([P, NVT, 2 * (DH + 1)], BF16)  # [tok,vt,130]
            resid_sb = persist.tile([P, NI2, D], F32)

            # ============ Phases 1+2 interleaved per batch ============
            ENGS = (nc.sync, nc.scalar, nc.gpsimd)
            NG = NJ // JG
            with (
                tc.tile_pool(name="xT", bufs=1) as xt_pool,
                tc.tile_pool(name="qkv_ps", bufs=2, space="PSUM") as qkv_ps,
                tc.tile_pool(name="sc_ps", bufs=1, space="PSUM") as sc_ps,
                tc.tile_pool(name="ctx_ps", bufs=1, space="PSUM") as ctx_pp,
                tc.tile_pool(name="e_pool", bufs=1) as e_pool,
                tc.tile_pool(name="attn_tmp", bufs=2) as attn_tmp,
            ):
                xt = {}   # (tensor_idx, c, b) -> tile
                ei = 0
                for b in range(B):
                    for ti, x_d in enumerate((xqT, xkT, xvT)):
                        for c in range(CCH):
                            t = xt_pool.tile([P, S], BF16,
                                             name=f"xT{ti}_{c}_{b}",
                                             tag="xT", bufs=16)
                            xt[(ti, c, b)] = t
                            ENGS[ei % 3].dma_start(
                                out=t, in_=x_d[c * P:(c + 1) * P,
                                               b * S:(b + 1) * S])
                            ei += 1
                for i2 in range(NI2):
                    (nc.sync if i2 % 2 else nc.scalar).dma_start(
                        out=resid_sb[:, i2, :],
                        in_=resid[i2 * P:(i2 + 1) * P, :])
                for c in range(CCH):
                    nc.scalar.dma_start(out=woT_sb[:, c, :],
                                        in_=woT[c * P:(c + 1) * P, :])

                for b in range(B):
                    tok0 = b * S
                    # ---- q/k projections for this batch ----
                    for ti, (w_sb, b_sb, o_sb) in enumerate(
                        ((wqT_sb, bq_sb, qT_sb), (wkT_sb, bk_sb, kT_sb))
                    ):
                        for n in range(S // 512):
                            ps = qkv_ps.tile([P, 512], F32, tag="proj")
                            for c in range(CCH):
                                nc.tensor.matmul(
                                    ps, w_sb[:, c, :],
                                    xt[(ti, c, b)][:, n * 512:(n + 1) * 512],
                                    start=(c == 0), stop=(c == CCH - 1))
                            nc.vector.tensor_scalar_add(
                                o_sb[:, tok0 + n * 512:tok0 + (n + 1) * 512],
                                ps, b_sb)
                    # ---- v projection for this batch ----
                    for itl in range(S // P):
                        it = b * (S // P) + itl
                        ps = qkv_ps.tile([P, DL], F32, tag="proj")
                        for c in range(CCH):
                            nc.tensor.matmul(
                                ps, xt[(2, c, b)][:, itl * P:(itl + 1) * P],
                                wvT_sb[:, c, :],
                                start=(c == 0), stop=(c == CCH - 1))
                        for h in range(HPC):
                            nc.vector.tensor_add(
                                v_sb[:, it, h * (DH + 1):h * (DH + 1) + DH],
                                ps[:, h * DH:(h + 1) * DH],
                                bv_bc[:, h * DH:(h + 1) * DH])
                            nc.vector.memset(
                                v_sb[:, it, h * (DH + 1) + DH:
                                     h * (DH + 1) + DH + 1], 1.0)

                    # ---- attention for this batch ----
                    for i in range(NI):
                        ic0 = tok0 + i * 512
                        e_t = [e_pool.tile([P, NG, JG * 512], BF16,
                                           name=f"e{h}_{b}_{i}", tag=f"e{h}",
                                           bufs=1)
                               for h in range(HPC)]
                        for g in range(NG):
                            ps = [sc_ps.tile([P, JG * 512], F32,
                                             name=f"scps{h}", tag=f"scps{h}",
                                             bufs=1)
                                  for h in range(HPC)]
                            for jj in range(JG):
                                j = g * JG + jj
                                jc0 = tok0 + j * P
                                for h in range(HPC):
                                    nc.tensor.matmul(
                                        ps[h][:, jj * 512:(jj + 1) * 512],
                                        kT_sb[h * DH:(h + 1) * DH,
                                              jc0:jc0 + P],
                                        qT_sb[h * DH:(h + 1) * DH,
                                              ic0:ic0 + 512],
                                        tile_position=(h * DH, 0))
                            for h in range(HPC):
                                nc.scalar.activation(
                                    e_t[h][:, g, :], ps[h],
                                    mybir.ActivationFunctionType.Exp)
                        for h in range(HPC):
                            cps = ctx_pp.tile([DH + 1, 512], F32,
                                              name=f"cps{h}", tag=f"cps{h}")
                            for j in range(NJ):
                                vt = b * (S // P) + j
                                nc.tensor.matmul(
                                    cps,
                                    v_sb[:, vt,
                                         h * (DH + 1):(h + 1) * (DH + 1)],
                                    e_t[h][:, j // JG,
                                           (j % JG) * 512:(j % JG + 1) * 512],
                                    start=(j == 0), stop=(j == NJ - 1))
                            ssum = attn_tmp.tile([1, 512], F32, tag="ssum")
                            nc.vector.tensor_copy(ssum, cps[DH:DH + 1, :])
                            rcp = attn_tmp.tile([1, 512], F32, tag="rcp")
                            nc.vector.reciprocal_approx_fast(rcp, ssum)
                            rcp_bc = attn_tmp.tile([DH, 512], F32, tag="rbc")
                            nc.gpsimd.partition_broadcast(rcp_bc, rcp)
                            ctxo = attn_tmp.tile([DH, 512], BF16,
                                                  tag="ctxo")
                            nc.vector.tensor_mul(ctxo, cps[0:DH, :], rcp_bc)
                            for nb in range(max(1, 512 // TPC)):
                                w = min(512, TPC)
                                nc.gpsimd.dma_start(
                                    out=a2a_in[(ic0 + nb * w) // TPC,
                                               h * DH:(h + 1) * DH,
                                               (ic0 + nb * w) % TPC:
                                               (ic0 + nb * w) % TPC + w],
                                    in_=ctxo[:, nb * w:(nb + 1) * w])
                        for nb in range(max(1, 512 // TPC)):
                            blk = (ic0 + nb * min(512, TPC)) // TPC
                            nc.gpsimd.collective_compute(
                                "AllGather", mybir.AluOpType.bypass,
                                replica_groups=[list(range(NCORES))],
                                ins=[a2a_in[blk].opt()],
                                outs=[ag_outs[blk].opt()])

            if debug_dumps:
                for nm, t in (("dbg_qT", qT_sb), ("dbg_kT", kT_sb),
                              ("dbg_v", v_sb)):
                    dout = nc.dram_tensor(nm, list(t.shape), BF16,
                                          kind="ExternalOutput").ap()
                    nc.sync.dma_start(out=dout, in_=t)

            # ================= Phase 3: out-proj + LN ============
            with (
                tc.tile_pool(name="op_ps", bufs=2, space="PSUM") as op_ps,
                tc.tile_pool(name="op_sb", bufs=2) as op_sb,
                tc.tile_pool(name="ctxF", bufs=1) as ctxf_pool,
            ):
                ctxF = ctxf_pool.tile([P, CCH, TPC], BF16)
                rk = nc.sync.cc_rank(replica_groups=[list(range(NCORES))])
                for c in range(CCH):
                    nc.sync.dma_start(
                        out=ctxF[:, c, :],
                        in_=ag_outs[bass.ds(rk, 1),
                                    c * P:(c + 1) * P, :])
                for i2 in range(NI2):
                    ps = op_ps.tile([P, D], F32, tag="op")
                    for c in range(CCH):
                        for n in range(D // 512):
                            nc.tensor.matmul(
                                ps[:, n * 512:(n + 1) * 512],
                                ctxF[:, c, i2 * P:(i2 + 1) * P],
                                woT_sb[:, c, n * 512:(n + 1) * 512],
                                start=(c == 0), stop=(c == CCH - 1))
                    y = op_sb.tile([P, D], F32, tag="y")
                    nc.vector.tensor_add(y, ps, bo_bc)
                    nc.vector.tensor_add(y, y, resid_sb[:, i2, :])
                    # LayerNorm
                    y3 = y.rearrange("p (g d) -> p g d", g=2)
                    stats = op_sb.tile([P, 2, 6], F32, tag="stats")
                    for g in range(2):
                        nc.vector.bn_stats(out=stats[:, g, :], in_=y3[:, g, :])
                    mv = op_sb.tile([P, 2], F32, tag="mv")
                    nc.vector.bn_aggr(out=mv, in_=stats)
                    std = op_sb.tile([P, 1], F32, tag="std")
                    nc.scalar.activation(std, mv[:, 1:2],
                                         mybir.ActivationFunctionType.Sqrt,
                                         bias=eps_sb)
                    rstd = op_sb.tile([P, 1], F32, tag="rstd")
                    nc.vector.reciprocal(rstd, std)
                    t32 = op_sb.tile([P, D], F32, tag="t32")
                    nc.vector.tensor_scalar(
                        out=t32, in0=y, scalar1=mv[:, 0:1], scalar2=rstd,
                        op0=mybir.AluOpType.subtract,
                        op1=mybir.AluOpType.mult)
                    of = op_sb.tile([P, D], F32, tag="of")
                    nc.vector.tensor_mul(of, t32, lnw_bc)
                    nc.vector.tensor_add(of, of, lnb_bc)
                    nc.sync.dma_start(out=out[i2 * P:(i2 + 1) * P, :], in_=of)

    nc.compile()
    return nc


_NC_CACHE = {}


def _get_nc(S=2048, B=2, D=1024):
    key = (S, B, D)
    if key not in _NC_CACHE:
        _NC_CACHE[key] = build_bert_kernel(S, B, D)
    return _NC_CACHE[key]


def make_in_maps(query_tensor, key_tensor, value_tensor, Wq, bq, Wk, bk,
                 Wv, bv, Wo, bo, ln_w, ln_b):
    S, B, D = query_tensor.shape
    NTOK = S * B
    TPC = NTOK // NCORES
    DL = (H // NCORES) * DH

    def bm(x):  # (S, B, D) -> batch-major (B*S, D) float32
        return np.ascontiguousarray(
            np.asarray(x, np.float32).transpose(1, 0, 2).reshape(NTOK, D))

    def bmT(x):  # feature-major bf16 (D, B*S)
        return np.ascontiguousarray(bm(x).T.astype(BF16_NP))

    xq = bm(query_tensor)
    xqT, xkT, xvT = bmT(query_tensor), bmT(key_tensor), bmT(value_tensor)
    woT = np.ascontiguousarray(
        np.asarray(Wo, np.float32).T.astype(BF16_NP))
    f32 = lambda a: np.ascontiguousarray(np.asarray(a, np.float32))
    bf16T = lambda a: np.ascontiguousarray(
        np.asarray(a, np.float32).T.astype(BF16_NP))
    in_maps = []
    for c in range(NCORES):
        sl = slice(c * DL, (c + 1) * DL)
        in_maps.append({
            "xqT": xqT, "xkT": xkT, "xvT": xvT,
            "wqT": bf16T(Wq[sl]), "wkT": bf16T(Wk[sl]),
            "wvT": bf16T(Wv[sl]), "woT": woT,
            "bq": f32(bq[sl]).reshape(DL, 1),
            "bk": f32(bk[sl]).reshape(DL, 1),
            "bv": f32(bv[sl]).reshape(1, DL),
            "bo": f32(bo).reshape(1, D),
            "lnw": f32(ln_w).reshape(1, D),
            "lnb": f32(ln_b).reshape(1, D),
            "resid": xq[c * TPC:(c + 1) * TPC],
        })
    return in_maps


def assemble_output(results, S, B, D):
    full = np.concatenate([r["out"] for r in results], axis=0)  # (B*S, D)
    return np.ascontiguousarray(
        full.reshape(B, S, D).transpose(1, 0, 2))


def kernel(**inputs):
    S, B, D = inputs["query_tensor"].shape
    nc = _get_nc(S, B, D)
    in_maps = make_in_maps(**inputs)
    res = run_bass_kernel_spmd(nc, in_maps, list(range(NCORES)))
    return assemble_output(res.results, S, B, D)


# revision 20
# speedup vs baseline: 1.0074x; 1.0074x over previous
"""BertAttention Trainium2 kernel — 8-core SPMD.

Sharding: each core owns 2 heads (128 of the 1024 feature dims).
  - QKV projections for its heads over all tokens (batch-major token order)
  - attention for its 4 (batch, head) pairs
  - AllToAll exchanges ctx^T slices -> each core holds all 1024 ctx dims
    for its 512-token slice
  - output projection (full Wo) + residual + LayerNorm on its token slice
  - host concatenates per-core (512, 1024) outputs.

Host passes activations/weights pre-transposed and pre-cast to bf16
(feature-major), so the device spends no time on casts/transposes.
"""

import os
import sys

for _p in ("/opt/trn_rl_repo", "/root/.axon_site/_ro/trn_rl_repo"):
    if os.path.isdir(_p) and _p not in sys.path:
        sys.path.append(_p)

import ml_dtypes
import numpy as np

import concourse.bass as bass
import concourse.tile as tile
from concourse import bacc, mybir
from concourse.bass_utils import run_bass_kernel_spmd

F32 = mybir.dt.float32
BF16 = mybir.dt.bfloat16
BF16_NP = ml_dtypes.bfloat16

NCORES = 8
H = 16  # heads total
DH = 64  # head dim
LN_EPS = 1e-12


def build_bert_kernel(S=2048, B=2, D=1024, debug_dumps=False):
    P = 128
    NTOK = S * B              # batch-major tokens
    TPC = NTOK // NCORES      # tokens per core (output slice)
    CCH = D // P              # contraction chunks (8)
    HPC = H // NCORES         # heads per core (2)
    DL = HPC * DH             # local feature dims (128)
    NI = S // 512             # i-chunks per batch (512 queries each)
    NJ = S // P               # j-chunks per batch (128 keys each)
    JG = 2                    # j-chunks per exp group
    NVT = NTOK // P           # v token tiles
    NI2 = TPC // P            # out-proj token tiles per core

    nc = bacc.Bacc("TRN2", target_bir_lowering=False, debug=False,
                   num_devices=NCORES)

    def din(name, shape, dt=F32):
        return nc.dram_tensor(name, list(shape), dt, kind="ExternalInput").ap()

    xqT = din("xqT", (D, NTOK), BF16)
    xkT = din("xkT", (D, NTOK), BF16)
    xvT = din("xvT", (D, NTOK), BF16)
    wqT = din("wqT", (D, DL), BF16)
    wkT = din("wkT", (D, DL), BF16)
    wvT = din("wvT", (D, DL), BF16)
    woT = din("woT", (D, D), BF16)
    bq = din("bq", (DL, 1))
    bk = din("bk", (DL, 1))
    bv = din("bv", (1, DL))
    bo = din("bo", (1, D))
    lnw = din("lnw", (1, D))
    lnb = din("lnb", (1, D))
    resid = din("resid", (TPC, D))
    out = nc.dram_tensor("out", [TPC, D], F32, kind="ExternalOutput").ap()

    a2a_in = nc.dram_tensor("a2a_in", [4, 2, P, TPC], BF16).ap()
    ag_outs = nc.dram_tensor("ag_outs", [4 * NCORES * 2, P, TPC], BF16).ap()

    with tile.TileContext(nc) as tc:
        with (
            tc.tile_pool(name="persist", bufs=1) as persist,
            tc.tile_pool(name="small", bufs=1) as small,
        ):
            # ---- weights into SBUF (plain DMA, already transposed) ----
            wqT_sb = persist.tile([P, CCH, DL], BF16)
            wkT_sb = persist.tile([P, CCH, DL], BF16)
            wvT_sb = persist.tile([P, CCH, DL], BF16)
            w_engs = (nc.sync, nc.scalar, nc.gpsimd)
            for wi, (w_d, w_sb) in enumerate(
                    ((wqT, wqT_sb), (wkT, wkT_sb), (wvT, wvT_sb))):
                for c in range(CCH):
                    w_engs[wi].dma_start(out=w_sb[:, c, :],
                                         in_=w_d[c * P:(c + 1) * P, :])
            woT_sb = persist.tile([P, CCH, D], BF16)

            # ---- constant / bias tiles ----
            bq_sb = small.tile([P, 1], F32)
            bk_sb = small.tile([P, 1], F32)
            nc.sync.dma_start(out=bq_sb, in_=bq)
            nc.sync.dma_start(out=bk_sb, in_=bk)
            bv_bc = small.tile([P, DL], F32)
            nc.gpsimd.dma_start(out=bv_bc, in_=bv.to_broadcast((P, DL)))
            bo_bc = small.tile([P, D], F32)
            nc.gpsimd.dma_start(out=bo_bc, in_=bo.to_broadcast((P, D)))
            lnw_bc = small.tile([P, D], F32)
            nc.gpsimd.dma_start(out=lnw_bc, in_=lnw.to_broadcast((P, D)))
            lnb_bc = small.tile([P, D], F32)
            nc.gpsimd.dma_start(out=lnb_bc, in_=lnb.to_broadcast((P, D)))
            eps_sb = small.tile([P, 1], F32)
            nc.vector.memset(eps_sb, LN_EPS)

            # ---- persistent activation buffers ----
            qT_sb = persist.tile([P, NTOK], BF16)   # [dloc, tok]
            kT_sb = persist.tile([P, NTOK], BF16)
            v_sb = persist.tile([P, NVT, 2 * (DH + 1)], BF16)  # [tok,vt,130]
            resid_sb = persist.tile([P, NI2, D], F32)

            # ============ Phases 1+2 interleaved per batch ============
            ENGS = (nc.sync, nc.scalar, nc.gpsimd)
            NG = NJ // JG
            with (
                tc.tile_pool(name="xT", bufs=1) as xt_pool,
                tc.tile_pool(name="qkv_ps", bufs=2, space="PSUM") as qkv_ps,
                tc.tile_pool(name="sc_ps", bufs=1, space="PSUM") as sc_ps,
                tc.tile_pool(name="ctx_ps", bufs=1, space="PSUM") as ctx_pp,
                tc.tile_pool(name="e_pool", bufs=1) as e_pool,
                tc.tile_pool(name="attn_tmp", bufs=2) as attn_tmp,
            ):
                xt = {}   # (tensor_idx, c, b) -> tile
                ei = 0
                for b in range(B):
                    for ti, x_d in enumerate((xqT, xkT, xvT)):
                        for c in range(CCH):
                            t = xt_pool.tile([P, S], BF16,
                                             name=f"xT{ti}_{c}_{b}",
                                             tag="xT", bufs=16)
                            xt[(ti, c, b)] = t
                            ENGS[ei % 3].dma_start(
                                out=t, in_=x_d[c * P:(c + 1) * P,
                                               b * S:(b + 1) * S])
                            ei += 1
                for i2 in range(NI2):
                    (nc.sync if i2 % 2 else nc.scalar).dma_start(
                        out=resid_sb[:, i2, :],
                        in_=resid[i2 * P:(i2 + 1) * P, :])
                for c in range(CCH):
                    nc.scalar.dma_start(out=woT_sb[:, c, :],
                                        in_=woT[c * P:(c + 1) * P, :])

                for b in range(B):
                    tok0 = b * S
                    # ---- q/k projections for this batch ----
                    for ti, (w_sb, b_sb, o_sb) in enumerate(
                        ((wqT_sb, bq_sb, qT_sb), (wkT_sb, bk_sb, kT_sb))
                    ):
                        for n in range(S // 512):
                            ps = qkv_ps.tile([P, 512], F32, tag="proj")
                            for c in range(CCH):
                                nc.tensor.matmul(
                                    ps, w_sb[:, c, :],
                                    xt[(ti, c, b)][:, n * 512:(n + 1) * 512],
                                    start=(c == 0), stop=(c == CCH - 1))
                            nc.vector.tensor_scalar_add(
                                o_sb[:, tok0 + n * 512:tok0 + (n + 1) * 512],
                                ps, b_sb)
                    # ---- v projection for this batch ----
                    for itl in range(S // P):
                        it = b * (S // P) + itl
                        ps = qkv_ps.tile([P, DL], F32, tag="proj")
                        for c in range(CCH):
                            nc.tensor.matmul(
                                ps, xt[(2, c, b)][:, itl * P:(itl + 1) * P],
                                wvT_sb[:, c, :],
                                start=(c == 0), stop=(c == CCH - 1))
                        for h in range(HPC):
                            nc.vector.tensor_add(
                                v_sb[:, it, h * (DH + 1):h * (DH + 1) + DH],
                                ps[:, h * DH:(h + 1) * DH],
                                bv_bc[:, h * DH:(h + 1) * DH])
                            nc.vector.memset(
                                v_sb[:, it, h * (DH + 1) + DH:
                                     h * (DH + 1) + DH + 1], 1.0)

                    # ---- attention for this batch ----
                    for i in range(NI):
                        ic0 = tok0 + i * 512
                        e_t = [e_pool.tile([P, NG, JG * 512], BF16,
                                           name=f"e{h}_{b}_{i}", tag=f"e{h}",
                                           bufs=1)
                               for h in range(HPC)]
                        for g in range(NG):
                            for h in range(HPC):
                                ps = sc_ps.tile([P, JG * 512], F32,
                                                name="scps", tag="scps",
                                                bufs=2)
                                for jj in range(JG):
                                    j = g * JG + jj
                                    jc0 = tok0 + j * P
                                    nc.tensor.matmul(
                                        ps[:, jj * 512:(jj + 1) * 512],
                                        kT_sb[h * DH:(h + 1) * DH,
                                              jc0:jc0 + P],
                                        qT_sb[h * DH:(h + 1) * DH,
                                              ic0:ic0 + 512])
                                nc.scalar.activation(
                                    e_t[h][:, g, :], ps,
                                    mybir.ActivationFunctionType.Exp)
                        for h in range(HPC):
                            cps = ctx_pp.tile([DH + 1, 512], F32,
                                              name=f"cps{h}", tag=f"cps{h}")
                            for j in range(NJ):
                                vt = b * (S // P) + j
                                nc.tensor.matmul(
                                    cps,
                                    v_sb[:, vt,
                                         h * (DH + 1):(h + 1) * (DH + 1)],
                                    e_t[h][:, j // JG,
                                           (j % JG) * 512:(j % JG + 1) * 512],
                                    start=(j == 0), stop=(j == NJ - 1))
                            ssum = attn_tmp.tile([1, 512], F32, tag="ssum")
                            nc.vector.tensor_copy(ssum, cps[DH:DH + 1, :])
                            rcp = attn_tmp.tile([1, 512], F32, tag="rcp")
                            nc.vector.reciprocal_approx_fast(rcp, ssum)
                            rcp_bc = attn_tmp.tile([DH, 512], F32, tag="rbc")
                            nc.gpsimd.partition_broadcast(rcp_bc, rcp)
                            ctxo = attn_tmp.tile([DH, 512], BF16,
                                                  tag="ctxo")
                            nc.vector.tensor_mul(ctxo, cps[0:DH, :], rcp_bc)
                            for nb in range(max(1, 512 // TPC)):
                                w = min(512, TPC)
                                blk = (ic0 + nb * w) // TPC
                                nc.gpsimd.dma_start(
                                    out=a2a_in[blk % 4, blk // 4,
                                               h * DH:(h + 1) * DH,
                                               (ic0 + nb * w) % TPC:
                                               (ic0 + nb * w) % TPC + w],
                                    in_=ctxo[:, nb * w:(nb + 1) * w])
                        for nb in range(max(1, 512 // TPC)):
                            blk = (ic0 + nb * min(512, TPC)) // TPC
                            if blk >= 4:
                                g = blk - 4
                                nc.gpsimd.collective_compute(
                                    "AllGather", mybir.AluOpType.bypass,
                                    replica_groups=[list(range(NCORES))],
                                    ins=[a2a_in[g].opt()],
                                    outs=[ag_outs[g * 16:(g + 1) * 16].opt()])

            if debug_dumps:
                for nm, t in (("dbg_qT", qT_sb), ("dbg_kT", kT_sb),
                              ("dbg_v", v_sb)):
                    dout = nc.dram_tensor(nm, list(t.shape), BF16,
                                          kind="ExternalOutput").ap()
                    nc.sync.dma_start(out=dout, in_=t)

            # ================= Phase 3: out-proj + LN ============
            with (
                tc.tile_pool(name="op_ps", bufs=2, space="PSUM") as op_ps,
                tc.tile_pool(name="op_sb", bufs=2) as op_sb,
                tc.tile_pool(name="ctxF", bufs=1) as ctxf_pool,
            ):
                ctxF = ctxf_pool.tile([P, CCH, TPC], BF16)
                rk = nc.sync.cc_rank(replica_groups=[list(range(NCORES))])
                base = (rk & 3) * 16 + (rk >> 2)
                for j in range(NCORES):
                    nc.sync.dma_start(
                        out=ctxF[:, j, :],
                        in_=ag_outs[bass.ds(base + j * 2, 1), :, :])
                for i2 in range(NI2):
                    ps = op_ps.tile([P, D], F32, tag="op")
                    for c in range(CCH):
                        for n in range(D // 512):
                            nc.tensor.matmul(
                                ps[:, n * 512:(n + 1) * 512],
                                ctxF[:, c, i2 * P:(i2 + 1) * P],
                                woT_sb[:, c, n * 512:(n + 1) * 512],
                                start=(c == 0), stop=(c == CCH - 1))
                    y = op_sb.tile([P, D], F32, tag="y")
                    nc.vector.tensor_add(y, ps, bo_bc)
                    nc.vector.tensor_add(y, y, resid_sb[:, i2, :])
                    # LayerNorm
                    y3 = y.rearrange("p (g d) -> p g d", g=2)
                    stats = op_sb.tile([P, 2, 6], F32, tag="stats")
                    for g in range(2):
                        nc.vector.bn_stats(out=stats[:, g, :], in_=y3[:, g, :])
                    mv = op_sb.tile([P, 2], F32, tag="mv")
                    nc.vector.bn_aggr(out=mv, in_=stats)
                    std = op_sb.tile([P, 1], F32, tag="std")
                    nc.scalar.activation(std, mv[:, 1:2],
                                         mybir.ActivationFunctionType.Sqrt,
                                         bias=eps_sb)
                    rstd = op_sb.tile([P, 1], F32, tag="rstd")
                    nc.vector.reciprocal(rstd, std)
                    t32 = op_sb.tile([P, D], F32, tag="t32")
                    nc.vector.tensor_scalar(
                        out=t32, in0=y, scalar1=mv[:, 0:1], scalar2=rstd,
                        op0=mybir.AluOpType.subtract,
                        op1=mybir.AluOpType.mult)
                    of = op_sb.tile([P, D], F32, tag="of")
                    nc.vector.tensor_mul(of, t32, lnw_bc)
                    nc.vector.tensor_add(of, of, lnb_bc)
                    nc.sync.dma_start(out=out[i2 * P:(i2 + 1) * P, :], in_=of)

    nc.compile()
    return nc


_NC_CACHE = {}


def _get_nc(S=2048, B=2, D=1024):
    key = (S, B, D)
    if key not in _NC_CACHE:
        _NC_CACHE[key] = build_bert_kernel(S, B, D)
    return _NC_CACHE[key]


def make_in_maps(query_tensor, key_tensor, value_tensor, Wq, bq, Wk, bk,
                 Wv, bv, Wo, bo, ln_w, ln_b):
    S, B, D = query_tensor.shape
    NTOK = S * B
    TPC = NTOK // NCORES
    DL = (H // NCORES) * DH

    def bm(x):  # (S, B, D) -> batch-major (B*S, D) float32
        return np.ascontiguousarray(
            np.asarray(x, np.float32).transpose(1, 0, 2).reshape(NTOK, D))

    def bmT(x):  # feature-major bf16 (D, B*S)
        return np.ascontiguousarray(bm(x).T.astype(BF16_NP))

    xq = bm(query_tensor)
    xqT, xkT, xvT = bmT(query_tensor), bmT(key_tensor), bmT(value_tensor)
    woT = np.ascontiguousarray(
        np.asarray(Wo, np.float32).T.astype(BF16_NP))
    f32 = lambda a: np.ascontiguousarray(np.asarray(a, np.float32))
    bf16T = lambda a: np.ascontiguousarray(
        np.asarray(a, np.float32).T.astype(BF16_NP))
    in_maps = []
    for c in range(NCORES):
        sl = slice(c * DL, (c + 1) * DL)
        in_maps.append({
            "xqT": xqT, "xkT": xkT, "xvT": xvT,
            "wqT": bf16T(Wq[sl]), "wkT": bf16T(Wk[sl]),
            "wvT": bf16T(Wv[sl]), "woT": woT,
            "bq": f32(bq[sl]).reshape(DL, 1),
            "bk": f32(bk[sl]).reshape(DL, 1),
            "bv": f32(bv[sl]).reshape(1, DL),
            "bo": f32(bo).reshape(1, D),
            "lnw": f32(ln_w).reshape(1, D),
            "lnb": f32(ln_b).reshape(1, D),
            "resid": xq[c * TPC:(c + 1) * TPC],
        })
    return in_maps


def assemble_output(results, S, B, D):
    full = np.concatenate([r["out"] for r in results], axis=0)  # (B*S, D)
    return np.ascontiguousarray(
        full.reshape(B, S, D).transpose(1, 0, 2))


def kernel(**inputs):
    S, B, D = inputs["query_tensor"].shape
    nc = _get_nc(S, B, D)
    in_maps = make_in_maps(**inputs)
    res = run_bass_kernel_spmd(nc, in_maps, list(range(NCORES)))
    return assemble_output(res.results, S, B, D)


# revision 21
# speedup vs baseline: 1.0595x; 1.0517x over previous
"""BertAttention Trainium2 kernel — 8-core SPMD.

Sharding: each core owns 2 heads (128 of the 1024 feature dims).
  - QKV projections for its heads over all tokens (batch-major token order)
  - attention for its 4 (batch, head) pairs
  - AllToAll exchanges ctx^T slices -> each core holds all 1024 ctx dims
    for its 512-token slice
  - output projection (full Wo) + residual + LayerNorm on its token slice
  - host concatenates per-core (512, 1024) outputs.

Host passes activations/weights pre-transposed and pre-cast to bf16
(feature-major), so the device spends no time on casts/transposes.
"""

import os
import sys

for _p in ("/opt/trn_rl_repo", "/root/.axon_site/_ro/trn_rl_repo"):
    if os.path.isdir(_p) and _p not in sys.path:
        sys.path.append(_p)

import ml_dtypes
import numpy as np

import concourse.bass as bass
import concourse.tile as tile
from concourse import bacc, mybir
from concourse.bass_utils import run_bass_kernel_spmd

F32 = mybir.dt.float32
BF16 = mybir.dt.bfloat16
BF16_NP = ml_dtypes.bfloat16

NCORES = 8
H = 16  # heads total
DH = 64  # head dim
LN_EPS = 1e-12


def build_bert_kernel(S=2048, B=2, D=1024, debug_dumps=False):
    P = 128
    NTOK = S * B              # batch-major tokens
    TPC = NTOK // NCORES      # tokens per core (output slice)
    CCH = D // P              # contraction chunks (8)
    HPC = H // NCORES         # heads per core (2)
    DL = HPC * DH             # local feature dims (128)
    NI = S // 512             # i-chunks per batch (512 queries each)
    NJ = S // P               # j-chunks per batch (128 keys each)
    JG = 2                    # j-chunks per exp group
    NVT = NTOK // P           # v token tiles
    NI2 = TPC // P            # out-proj token tiles per core

    nc = bacc.Bacc("TRN2", target_bir_lowering=False, debug=False,
                   num_devices=NCORES)

    def din(name, shape, dt=F32):
        return nc.dram_tensor(name, list(shape), dt, kind="ExternalInput").ap()

    xqT = din("xqT", (D, NTOK), BF16)
    xkT = din("xkT", (D, NTOK), BF16)
    xvT = din("xvT", (D, NTOK), BF16)
    wqT = din("wqT", (D, DL), BF16)
    wkT = din("wkT", (D, DL), BF16)
    wvT = din("wvT", (D, DL), BF16)
    woT = din("woT", (D, D), BF16)
    bq = din("bq", (DL, 1))
    bk = din("bk", (DL, 1))
    bv = din("bv", (1, DL))
    bo = din("bo", (1, D))
    lnw = din("lnw", (1, D))
    lnb = din("lnb", (1, D))
    resid = din("resid", (TPC, D))
    out = nc.dram_tensor("out", [TPC, D], F32, kind="ExternalOutput").ap()

    NBLK = NTOK // TPC
    a2a_in = nc.dram_tensor("a2a_in", [NBLK, P, TPC], BF16).ap()
    ag_outs = nc.dram_tensor("ag_outs", [NBLK * NCORES, P, TPC], BF16).ap()

    with tile.TileContext(nc) as tc:
        with (
            tc.tile_pool(name="persist", bufs=1) as persist,
            tc.tile_pool(name="small", bufs=1) as small,
        ):
            # ---- weights into SBUF (plain DMA, already transposed) ----
            wqT_sb = persist.tile([P, CCH, DL], BF16)
            wkT_sb = persist.tile([P, CCH, DL], BF16)
            wvT_sb = persist.tile([P, CCH, DL], BF16)
            w_engs = (nc.sync, nc.scalar, nc.gpsimd)
            for wi, (w_d, w_sb) in enumerate(
                    ((wqT, wqT_sb), (wkT, wkT_sb), (wvT, wvT_sb))):
                for c in range(CCH):
                    w_engs[wi].dma_start(out=w_sb[:, c, :],
                                         in_=w_d[c * P:(c + 1) * P, :])
            woT_sb = persist.tile([P, CCH, D], BF16)

            # ---- constant / bias tiles ----
            bq_sb = small.tile([P, 1], F32)
            bk_sb = small.tile([P, 1], F32)
            nc.sync.dma_start(out=bq_sb, in_=bq)
            nc.sync.dma_start(out=bk_sb, in_=bk)
            bv_bc = small.tile([P, DL], F32)
            nc.gpsimd.dma_start(out=bv_bc, in_=bv.to_broadcast((P, DL)))
            bo_bc = small.tile([P, D], F32)
            nc.gpsimd.dma_start(out=bo_bc, in_=bo.to_broadcast((P, D)))
            lnw_bc = small.tile([P, D], F32)
            nc.gpsimd.dma_start(out=lnw_bc, in_=lnw.to_broadcast((P, D)))
            lnb_bc = small.tile([P, D], F32)
            nc.gpsimd.dma_start(out=lnb_bc, in_=lnb.to_broadcast((P, D)))
            eps_sb = small.tile([P, 1], F32)
            nc.vector.memset(eps_sb, LN_EPS)

            # ---- persistent activation buffers ----
            qT_sb = persist.tile([P, NTOK], BF16)   # [dloc, tok]
            kT_sb = persist.tile([P, NTOK], BF16)
            v_sb = persist.tile([P, NVT, 2 * (DH + 1)], BF16)  # [tok,vt,130]
            resid_sb = persist.tile([P, NI2, D], F32)

            # ============ Phases 1+2 interleaved per batch ============
            ENGS = (nc.sync, nc.scalar, nc.gpsimd)
            NG = NJ // JG
            with (
                tc.tile_pool(name="xT", bufs=1) as xt_pool,
                tc.tile_pool(name="qkv_ps", bufs=2, space="PSUM") as qkv_ps,
                tc.tile_pool(name="sc_ps", bufs=1, space="PSUM") as sc_ps,
                tc.tile_pool(name="ctx_ps", bufs=1, space="PSUM") as ctx_pp,
                tc.tile_pool(name="e_pool", bufs=1) as e_pool,
                tc.tile_pool(name="attn_tmp", bufs=2) as attn_tmp,
            ):
                xt = {}   # (tensor_idx, c, b) -> tile
                ei = 0
                for b in range(B):
                    for ti, x_d in enumerate((xqT, xkT, xvT)):
                        for c in range(CCH):
                            t = xt_pool.tile([P, S], BF16,
                                             name=f"xT{ti}_{c}_{b}",
                                             tag="xT", bufs=16)
                            xt[(ti, c, b)] = t
                            ENGS[ei % 3].dma_start(
                                out=t, in_=x_d[c * P:(c + 1) * P,
                                               b * S:(b + 1) * S])
                            ei += 1
                for i2 in range(NI2):
                    (nc.sync if i2 % 2 else nc.scalar).dma_start(
                        out=resid_sb[:, i2, :],
                        in_=resid[i2 * P:(i2 + 1) * P, :])
                for c in range(CCH):
                    nc.scalar.dma_start(out=woT_sb[:, c, :],
                                        in_=woT[c * P:(c + 1) * P, :])

                for b in range(B):
                    tok0 = b * S
                    # ---- q/k projections for this batch ----
                    for ti, (w_sb, b_sb, o_sb) in enumerate(
                        ((wqT_sb, bq_sb, qT_sb), (wkT_sb, bk_sb, kT_sb))
                    ):
                        for n in range(S // 512):
                            ps = qkv_ps.tile([P, 512], F32, tag="proj")
                            for c in range(CCH):
                                nc.tensor.matmul(
                                    ps, w_sb[:, c, :],
                                    xt[(ti, c, b)][:, n * 512:(n + 1) * 512],
                                    start=(c == 0), stop=(c == CCH - 1))
                            nc.vector.tensor_scalar_add(
                                o_sb[:, tok0 + n * 512:tok0 + (n + 1) * 512],
                                ps, b_sb)
                    # ---- v projection for this batch ----
                    for itl in range(S // P):
                        it = b * (S // P) + itl
                        ps = qkv_ps.tile([P, DL], F32, tag="proj")
                        for c in range(CCH):
                            nc.tensor.matmul(
                                ps, xt[(2, c, b)][:, itl * P:(itl + 1) * P],
                                wvT_sb[:, c, :],
                                start=(c == 0), stop=(c == CCH - 1))
                        for h in range(HPC):
                            nc.vector.tensor_add(
                                v_sb[:, it, h * (DH + 1):h * (DH + 1) + DH],
                                ps[:, h * DH:(h + 1) * DH],
                                bv_bc[:, h * DH:(h + 1) * DH])
                            nc.vector.memset(
                                v_sb[:, it, h * (DH + 1) + DH:
                                     h * (DH + 1) + DH + 1], 1.0)

                    # ---- attention for this batch ----
                    for i in range(NI):
                        ic0 = tok0 + i * 512
                        e_t = [e_pool.tile([P, NG, JG * 512], BF16,
                                           name=f"e{h}_{b}_{i}", tag=f"e{h}",
                                           bufs=1)
                               for h in range(HPC)]
                        for g in range(NG):
                            for h in range(HPC):
                                ps = sc_ps.tile([P, JG * 512], F32,
                                                name="scps", tag="scps",
                                                bufs=2)
                                for jj in range(JG):
                                    j = g * JG + jj
                                    jc0 = tok0 + j * P
                                    nc.tensor.matmul(
                                        ps[:, jj * 512:(jj + 1) * 512],
                                        kT_sb[h * DH:(h + 1) * DH,
                                              jc0:jc0 + P],
                                        qT_sb[h * DH:(h + 1) * DH,
                                              ic0:ic0 + 512])
                                nc.scalar.activation(
                                    e_t[h][:, g, :], ps,
                                    mybir.ActivationFunctionType.Exp)
                        for h in range(HPC):
                            cps = ctx_pp.tile([DH + 1, 512], F32,
                                              name=f"cps{h}", tag=f"cps{h}")
                            for j in range(NJ):
                                vt = b * (S // P) + j
                                nc.tensor.matmul(
                                    cps,
                                    v_sb[:, vt,
                                         h * (DH + 1):(h + 1) * (DH + 1)],
                                    e_t[h][:, j // JG,
                                           (j % JG) * 512:(j % JG + 1) * 512],
                                    start=(j == 0), stop=(j == NJ - 1))
                            ssum = attn_tmp.tile([1, 512], F32, tag="ssum")
                            nc.vector.tensor_copy(ssum, cps[DH:DH + 1, :])
                            rcp = attn_tmp.tile([1, 512], F32, tag="rcp")
                            nc.vector.reciprocal_approx_fast(rcp, ssum)
                            rcp_bc = attn_tmp.tile([DH, 512], F32, tag="rbc")
                            nc.gpsimd.partition_broadcast(rcp_bc, rcp)
                            ctxo = attn_tmp.tile([DH, 512], BF16,
                                                  tag="ctxo")
                            nc.vector.tensor_mul(ctxo, cps[0:DH, :], rcp_bc)
                            for nb in range(max(1, 512 // TPC)):
                                w = min(512, TPC)
                                blk = (ic0 + nb * w) // TPC
                                nc.gpsimd.dma_start(
                                    out=a2a_in[blk,
                                               h * DH:(h + 1) * DH,
                                               (ic0 + nb * w) % TPC:
                                               (ic0 + nb * w) % TPC + w],
                                    in_=ctxo[:, nb * w:(nb + 1) * w])
                        half = NBLK // 2
                        for nb in range(max(1, 512 // TPC)):
                            blk = (ic0 + nb * min(512, TPC)) // TPC
                            if blk == half - 1:
                                # batch-0 blocks in one early AllGather
                                nc.gpsimd.collective_compute(
                                    "AllGather", mybir.AluOpType.bypass,
                                    replica_groups=[list(range(NCORES))],
                                    ins=[a2a_in[0:half].opt()],
                                    outs=[ag_outs[0:half * NCORES].opt()])
                            elif blk >= half:
                                g = blk - half
                                o0 = half * NCORES + g * NCORES
                                nc.gpsimd.collective_compute(
                                    "AllGather", mybir.AluOpType.bypass,
                                    replica_groups=[list(range(NCORES))],
                                    ins=[a2a_in[blk].opt()],
                                    outs=[ag_outs[o0:o0 + NCORES].opt()])

            if debug_dumps:
                for nm, t in (("dbg_qT", qT_sb), ("dbg_kT", kT_sb),
                              ("dbg_v", v_sb)):
                    dout = nc.dram_tensor(nm, list(t.shape), BF16,
                                          kind="ExternalOutput").ap()
                    nc.sync.dma_start(out=dout, in_=t)

            # ================= Phase 3: out-proj + LN ============
            with (
                tc.tile_pool(name="op_ps", bufs=2, space="PSUM") as op_ps,
                tc.tile_pool(name="op_sb", bufs=2) as op_sb,
                tc.tile_pool(name="ctxF", bufs=1) as ctxf_pool,
            ):
                ctxF = ctxf_pool.tile([P, CCH, TPC], BF16)
                rk = nc.sync.cc_rank(replica_groups=[list(range(NCORES))])
                half = NBLK // 2
                m = rk >> 2  # 0 for batch-0 ranks, 1 for batch-1 ranks
                for j in range(NCORES):
                    # rk < half: record j*half + rk (big AG, rank-major)
                    # rk >= half: record half*8 + (rk-half)*8 + j = 8*rk + j
                    off = (1 - m) * (rk + half * j) + m * (rk * NCORES + j)
                    nc.sync.dma_start(
                        out=ctxF[:, j, :],
                        in_=ag_outs[bass.ds(off, 1), :, :])
                for i2 in range(NI2):
                    ps = op_ps.tile([P, D], F32, tag="op")
                    for c in range(CCH):
                        for n in range(D // 512):
                            nc.tensor.matmul(
                                ps[:, n * 512:(n + 1) * 512],
                                ctxF[:, c, i2 * P:(i2 + 1) * P],
                                woT_sb[:, c, n * 512:(n + 1) * 512],
                                start=(c == 0), stop=(c == CCH - 1))
                    y = op_sb.tile([P, D], F32, tag="y")
                    nc.vector.tensor_add(y, ps, bo_bc)
                    nc.vector.tensor_add(y, y, resid_sb[:, i2, :])
                    # LayerNorm
                    y3 = y.rearrange("p (g d) -> p g d", g=2)
                    stats = op_sb.tile([P, 2, 6], F32, tag="stats")
                    for g in range(2):
                        nc.vector.bn_stats(out=stats[:, g, :], in_=y3[:, g, :])
                    mv = op_sb.tile([P, 2], F32, tag="mv")
                    nc.vector.bn_aggr(out=mv, in_=stats)
                    std = op_sb.tile([P, 1], F32, tag="std")
                    nc.scalar.activation(std, mv[:, 1:2],
                                         mybir.ActivationFunctionType.Sqrt,
                                         bias=eps_sb)
                    rstd = op_sb.tile([P, 1], F32, tag="rstd")
                    nc.vector.reciprocal(rstd, std)
                    t32 = op_sb.tile([P, D], F32, tag="t32")
                    nc.vector.tensor_scalar(
                        out=t32, in0=y, scalar1=mv[:, 0:1], scalar2=rstd,
                        op0=mybir.AluOpType.subtract,
                        op1=mybir.AluOpType.mult)
                    of = op_sb.tile([P, D], F32, tag="of")
                    nc.vector.tensor_mul(of, t32, lnw_bc)
                    nc.vector.tensor_add(of, of, lnb_bc)
                    nc.sync.dma_start(out=out[i2 * P:(i2 + 1) * P, :], in_=of)

    nc.compile()
    return nc


_NC_CACHE = {}


def _get_nc(S=2048, B=2, D=1024):
    key = (S, B, D)
    if key not in _NC_CACHE:
        _NC_CACHE[key] = build_bert_kernel(S, B, D)
    return _NC_CACHE[key]


def make_in_maps(query_tensor, key_tensor, value_tensor, Wq, bq, Wk, bk,
                 Wv, bv, Wo, bo, ln_w, ln_b):
    S, B, D = query_tensor.shape
    NTOK = S * B
    TPC = NTOK // NCORES
    DL = (H // NCORES) * DH

    def bm(x):  # (S, B, D) -> batch-major (B*S, D) float32
        return np.ascontiguousarray(
            np.asarray(x, np.float32).transpose(1, 0, 2).reshape(NTOK, D))

    def bmT(x):  # feature-major bf16 (D, B*S)
        return np.ascontiguousarray(bm(x).T.astype(BF16_NP))

    xq = bm(query_tensor)
    xqT, xkT, xvT = bmT(query_tensor), bmT(key_tensor), bmT(value_tensor)
    woT = np.ascontiguousarray(
        np.asarray(Wo, np.float32).T.astype(BF16_NP))
    f32 = lambda a: np.ascontiguousarray(np.asarray(a, np.float32))
    bf16T = lambda a: np.ascontiguousarray(
        np.asarray(a, np.float32).T.astype(BF16_NP))
    in_maps = []
    for c in range(NCORES):
        sl = slice(c * DL, (c + 1) * DL)
        in_maps.append({
            "xqT": xqT, "xkT": xkT, "xvT": xvT,
            "wqT": bf16T(Wq[sl]), "wkT": bf16T(Wk[sl]),
            "wvT": bf16T(Wv[sl]), "woT": woT,
            "bq": f32(bq[sl]).reshape(DL, 1),
            "bk": f32(bk[sl]).reshape(DL, 1),
            "bv": f32(bv[sl]).reshape(1, DL),
            "bo": f32(bo).reshape(1, D),
            "lnw": f32(ln_w).reshape(1, D),
            "lnb": f32(ln_b).reshape(1, D),
            "resid": xq[c * TPC:(c + 1) * TPC],
        })
    return in_maps


def assemble_output(results, S, B, D):
    full = np.concatenate([r["out"] for r in results], axis=0)  # (B*S, D)
    return np.ascontiguousarray(
        full.reshape(B, S, D).transpose(1, 0, 2))


def kernel(**inputs):
    S, B, D = inputs["query_tensor"].shape
    nc = _get_nc(S, B, D)
    in_maps = make_in_maps(**inputs)
    res = run_bass_kernel_spmd(nc, in_maps, list(range(NCORES)))
    return assemble_output(res.results, S, B, D)
